# revision 2
# baseline (speedup 1.0000x reference)
"""GCNConv (PyG-style, alpha-blended residual) on 8 Trainium2 NeuronCores.

Strategy (graph/data parallel, zero collectives):
  out = a*x + (1-a)*(Ahat @ x @ W.T + b)        (aggregate-first form)

v2 redesign (310us -> target ~150us), driven by trace analysis of v1:
  - v1 bottleneck was the dma_gather path: each SWDGE queue serialized
    gen+transfer+sem per ~1280-row call (only ~1 call in flight per queue,
    descriptor in-flight cap ~128/queue at ndesc=rows/16+1), so aggregate
    gather BW averaged 177GB/s vs 347GB/s instantaneous peak. v2 uses
    smaller calls (<=4 chunks = 512 rows, ndesc 33) so 2-3 calls pipeline
    per queue, keeping transfers back-to-back.
  - fp8(e4m3) gather table (dinv[src]-prescaled xs rows, 256B/row): halves
    the dominant DMA stream (53MB -> 29.9MB/core). Numpy-validated final
    rel err ~0.7% vs the 2e-2 gate.
  - Identity selection instead of per-slot DVE S-builds: destination nodes
    are PERMUTED so each group of 128 dsts has near-equal in-degree
    (global (h0,h1) sort) and each source is 2-COLORED into the two
    int16-index table halves so every dst's in-edges split evenly
    (greedy discrepancy minimization). Chunk c of a slot then holds the
    c-th edge of every dst: lane index == dst offset, the selection
    matrix is a constant fp8 identity, holes gather a dedicated zero row
    (9.9% overhead). No S-builds (DVE freed), no dof table, no memsets
    (matmuls only touch gathered chunks).
  - Self-loops are ordinary edges in the gather (src==dst, prescaled
    table makes dinv^2*x automatic): slab2 path of v1 deleted.
  - fp8 DoubleRow PE matmuls aggregate 2 chunks (256-deep contraction)
    per instruction with a stacked-identity lhsT.
  - xres preblended residual in bf16; out fp32; W-chain bf16 as in v1.
Degrees / normalization / coloring / packing are static graph
preprocessing done host-side (pure numpy). NOTE from v1: idx=-1
trailing-trim and bf16 PSUM transposes hang the device - do not
reintroduce.
"""

import numpy as np

import concourse.bacc as bacc
import concourse.bass as bass
import concourse.mybir as mybir
import concourse.tile as tile
from concourse.bass_utils import run_bass_kernel_spmd

N_NODES = 50000
D = 256
M_CORES = 8
P = 128
HALF = 25000
ZROW = HALF  # zero-row index in each half table
NG = (N_NODES + P - 1) // P         # 391 dst groups (permuted)
SLOTS = (NG + M_CORES - 1) // M_CORES  # 49 slots per core
MAX_CALL = 4                        # chunks per dma_gather call (pipelining)

F32 = mybir.dt.float32
BF16 = mybir.dt.bfloat16
FP8 = mybir.dt.float8e4
I16 = mybir.dt.int16

NQ = 4                              # SWDGE queues (Q7 core pairs)


def _split_call(c):
    out = []
    while c > 0:
        take = min(MAX_CALL, c)
        out.append(take)
        c -= take
    return out


def _color_sources(srcA, dstA, n):
    """Balanced 2-coloring of sources minimizing per-dst half imbalance."""
    order_by_src = np.argsort(srcA, kind="stable")
    dd = dstA[order_by_src]
    ss = srcA[order_by_src]
    starts = np.searchsorted(ss, np.arange(n + 1))

    rng = np.random.default_rng(0)
    color = np.zeros(n, dtype=np.int8)
    imb = np.zeros(n, dtype=np.int64)  # h0-h1 per dst (assigned so far)
    cnt = [0, 0]
    cap = [HALF, n - HALF]
    proc = rng.permutation(n)
    for s in proc:
        ds = dd[starts[s]:starts[s + 1]]
        sc = imb[ds].sum()
        c = 0 if sc < 0 else 1 if sc > 0 else (0 if cnt[0] <= cnt[1] else 1)
        if cnt[c] >= cap[c]:
            c = 1 - c
        color[s] = c
        cnt[c] += 1
        imb[ds] += 1 if c == 0 else -1
    for _ in range(2):  # refinement
        for s in proc:
            ds = dd[starts[s]:starts[s + 1]]
            sgn = 1 if color[s] == 0 else -1
            if np.sum((imb[ds] - 2 * sgn) ** 2 - imb[ds] ** 2) < 0:
                c = color[s]
                color[s] = 1 - c
                cnt[c] -= 1
                cnt[1 - c] += 1
                imb[ds] -= 2 * sgn
    # restore exact 25000/25000 with least-damaging flips
    over = 0 if cnt[0] > HALF else 1
    sgn = 1 if over == 0 else -1
    while cnt[over] > (HALF if over == 0 else n - HALF):
        cands = np.where(color == over)[0]
        sel = cands if len(cands) <= 4000 else rng.choice(cands, 4000, replace=False)
        best, bestd = None, None
        for s in sel:
            ds = dd[starts[s]:starts[s + 1]]
            dl = np.sum((imb[ds] - 2 * sgn) ** 2 - imb[ds] ** 2)
            if bestd is None or dl < bestd:
                best, bestd = s, dl
        ds = dd[starts[best]:starts[best + 1]]
        color[best] = 1 - over
        cnt[over] -= 1
        cnt[1 - over] += 1
        imb[ds] -= 2 * sgn
    return color


def _preprocess(node_features, edge_index, W, b, alpha):
    x = np.ascontiguousarray(np.asarray(node_features, dtype=np.float32))
    ei = np.asarray(edge_index)
    a = float(np.asarray(alpha).reshape(-1)[0])
    Wf = np.asarray(W, dtype=np.float32)
    bf = np.asarray(b, dtype=np.float32)

    src0 = ei[0].astype(np.int64)
    dst0 = ei[1].astype(np.int64)
    # self-loops as ordinary edges
    srcA = np.concatenate([src0, np.arange(N_NODES)])
    dstA = np.concatenate([dst0, np.arange(N_NODES)])

    deg = np.bincount(dstA, minlength=N_NODES).astype(np.float32)
    dinv = (1.0 / np.sqrt(deg)).astype(np.float32)
    xs = dinv[:, None] * x                      # prescaled gather rows

    # --- source coloring -> table halves -------------------------------
    color = _color_sources(srcA, dstA, N_NODES)
    # table position within half: color-0 nodes by id -> 0..24999, etc.
    pos_in_half = np.zeros(N_NODES, dtype=np.int64)
    for c in (0, 1):
        nodes = np.where(color == c)[0]
        pos_in_half[nodes] = np.arange(len(nodes))
    halfb = color[srcA].astype(np.int64)        # per-edge half
    eidx = pos_in_half[srcA]                    # per-edge idx within half

    fp8 = mybir.dt.np(FP8)
    tabs = []
    for c in (0, 1):
        nodes = np.where(color == c)[0]
        t = np.zeros((HALF + 1, D), dtype=fp8)
        t[:len(nodes)] = xs[nodes].astype(fp8)
        tabs.append(t)                          # row ZROW stays zero

    # --- dst permutation: group by (h0, h1) -----------------------------
    h1 = np.bincount(dstA[halfb == 1], minlength=N_NODES)
    h0 = deg.astype(np.int64) - h1
    order = np.lexsort((h1, h0))                # nodes sorted by (h0, h1)
    C0g = np.zeros(NG, dtype=np.int64)
    C1g = np.zeros(NG, dtype=np.int64)
    for g in range(NG):
        blk = order[g * P:(g + 1) * P]
        C0g[g] = h0[blk].max() if len(blk) else 0
        C1g[g] = h1[blk].max() if len(blk) else 0

    # deal groups 8 per slot row, packing equal (C0,C1) classes together
    gorder = np.argsort(-(C0g * 100 + C1g), kind="stable")
    assign = np.full((M_CORES, SLOTS), -1, dtype=np.int64)
    C0r = np.zeros(SLOTS, dtype=np.int64)
    C1r = np.zeros(SLOTS, dtype=np.int64)
    for r in range(SLOTS):
        blk = gorder[r * M_CORES:(r + 1) * M_CORES]
        for c, g in enumerate(blk):
            assign[c, r] = g
        C0r[r] = C0g[blk].max()
        C1r[r] = C1g[blk].max()
    Cr = C0r + C1r
    cofs = np.concatenate([[0], np.cumsum(Cr)[:-1]])
    TOT = int(Cr.sum())

    # node -> (group, lane); group -> (core, slot)
    grp_of = np.zeros(N_NODES, dtype=np.int64)
    lane_of = np.zeros(N_NODES, dtype=np.int64)
    grp_of[order] = np.arange(N_NODES) // P
    lane_of[order] = np.arange(N_NODES) % P
    core_of_g = np.zeros(NG, dtype=np.int64)
    slot_of_g = np.zeros(NG, dtype=np.int64)
    for c in range(M_CORES):
        for r in range(SLOTS):
            g = assign[c, r]
            if g >= 0:
                core_of_g[g] = c
                slot_of_g[g] = r

    # --- per-edge placement: k-th half-h edge of dst -> chunk k ---------
    # rank edges within (dst, half)
    ekey = dstA * 2 + halfb
    eorder = np.argsort(ekey, kind="stable")
    ks = ekey[eorder]
    kstarts = np.concatenate([[0], np.cumsum(np.bincount(ks, minlength=2 * N_NODES))[:-1]])
    rank = np.arange(len(ks)) - kstarts[ks]     # k within (dst, half)

    ed = dstA[eorder]
    eh = halfb[eorder]
    eix = eidx[eorder]
    g_e = grp_of[ed]
    cr_e = core_of_g[g_e]
    sl_e = slot_of_g[g_e]
    chunk_e = cofs[sl_e] + np.where(eh == 1, C0r[sl_e], 0) + rank
    gpos = chunk_e * P + lane_of[ed]

    idx_arr = np.full((M_CORES, TOT * P), ZROW, dtype=np.int16)  # holes
    idx_arr[cr_e, gpos] = eix.astype(np.int16)

    gidx = [
        np.tile(idx_arr[c].reshape(TOT * 8, 16).T, (8, 1)) for c in range(M_CORES)
    ]

    # --- per-(core,slot) dinv + preblended residual ---------------------
    bf16 = mybir.dt.np(BF16)
    xres_sl = []
    dinv_sl = []
    for c in range(M_CORES):
        slab = np.zeros((P, SLOTS, D), dtype=np.float32)
        dslab = np.zeros((P, SLOTS), dtype=np.float32)
        for r in range(SLOTS):
            g = assign[c, r]
            if g < 0:
                continue
            blk = order[g * P:(g + 1) * P]
            n = len(blk)
            slab[:n, r, :] = a * x[blk] + (1.0 - a) * bf[None, :]
            dslab[:n, r] = dinv[blk]
        xres_sl.append(np.ascontiguousarray(slab.astype(bf16)))
        dinv_sl.append(np.ascontiguousarray(dslab))

    wtp = np.ascontiguousarray(((1.0 - a) * Wf.T).astype(np.float32))
    ident2 = np.concatenate([np.eye(P, dtype=fp8)] * 2, axis=1)  # [P, 2P]
    ident = np.eye(P, dtype=np.float32)

    meta = dict(C0r=C0r, C1r=C1r, cofs=cofs, TOT=TOT, assign=assign, order=order)
    return tabs, gidx, xres_sl, dinv_sl, wtp, ident2, ident, meta


def _build(meta):
    C0r, C1r, cofs, TOT = meta["C0r"], meta["C1r"], meta["cofs"], meta["TOT"]
    nc = bacc.Bacc("TRN2", debug=False, num_swdge_queues=NQ, use_seq_codegen=True)

    xtab0 = nc.dram_tensor("xtab0", [HALF + 1, D], FP8, kind="ExternalInput")
    xtab1 = nc.dram_tensor("xtab1", [HALF + 1, D], FP8, kind="ExternalInput")
    xres = nc.dram_tensor("xres", [P, SLOTS * D], BF16, kind="ExternalInput")
    gidx = nc.dram_tensor("gidx", [P, TOT * 8], I16, kind="ExternalInput")
    dinvv = nc.dram_tensor("dinvv", [P, SLOTS], F32, kind="ExternalInput")
    wtp = nc.dram_tensor("wtp", [2 * P, D], BF16, kind="ExternalInput")
    ident2 = nc.dram_tensor("ident2", [P, 2 * P], FP8, kind="ExternalInput")
    ident = nc.dram_tensor("ident", [P, P], F32, kind="ExternalInput")
    out = nc.dram_tensor("out", [P, SLOTS * D], F32, kind="ExternalOutput")
    BAT = 7                             # slots per xres/out DMA batch

    with tile.TileContext(nc) as tc:
        with (
            tc.tile_pool(name="const", bufs=1) as cpool,
            tc.tile_pool(name="xg", bufs=10) as xg_pool,
            tc.tile_pool(name="sb", bufs=3) as sb_pool,
            tc.tile_pool(name="io", bufs=3) as io_pool,
            tc.tile_pool(name="pagg", bufs=3, space="PSUM") as pagg_pool,
            tc.tile_pool(name="pt", bufs=2, space="PSUM") as pt_pool,
            tc.tile_pool(name="pout", bufs=2, space="PSUM") as pout_pool,
        ):
            # slot-0 index columns first: tiny DMA so gathers start early
            s0c = int(cofs[1]) * 8
            gidx0_sb = cpool.tile([P, s0c], I16)
            gidxR_sb = cpool.tile([P, TOT * 8 - s0c], I16)
            ident2_sb = cpool.tile([P, 2 * P], FP8)
            ident_sb = cpool.tile([P, P], F32)
            wtp0_sb = cpool.tile([P, D], BF16)
            wtp1_sb = cpool.tile([P, D], BF16)
            dinv_sb = cpool.tile([P, SLOTS], F32)
            nc.sync.dma_start(out=gidx0_sb[:], in_=gidx[:, 0:s0c])
            nc.sync.dma_start(out=ident2_sb[:], in_=ident2[:])
            nc.sync.dma_start(out=ident_sb[:], in_=ident[:])
            nc.sync.dma_start(out=wtp0_sb[:], in_=wtp[0:P, :])
            nc.sync.dma_start(out=wtp1_sb[:], in_=wtp[P:2 * P, :])
            nc.sync.dma_start(out=dinv_sb[:], in_=dinvv[:])
            nc.sync.dma_start(out=gidxR_sb[:], in_=gidx[:, s0c:TOT * 8])

            CMAX = int((C0r + C1r).max())

            qrr = 0  # round-robin SWDGE queue over Q7 core pairs
            xres_sb = None
            out_sb = None
            for r in range(SLOTS):
                C0, C1 = int(C0r[r]), int(C1r[r])
                C = C0 + C1
                co = int(cofs[r])
                j = r % BAT
                if j == 0:
                    nb = min(BAT, SLOTS - r)
                    xres_sb = io_pool.tile([P, BAT, D], BF16, tag="xres")
                    nc.scalar.dma_start(
                        out=xres_sb[:, 0:nb, :],
                        in_=xres[:, r * D:(r + nb) * D].rearrange(
                            "p (b d) -> p b d", b=nb
                        ),
                    )
                    out_sb = io_pool.tile([P, BAT, D], F32, tag="out")

                xg = xg_pool.tile([P, CMAX, D], FP8, tag="xg")
                cc0 = 0
                for tab_ap, n_chunks in ((xtab0[:, :], C0), (xtab1[:, :], C1)):
                    for n_ch in _split_call(n_chunks):
                        ni = n_ch * P
                        if r == 0:
                            gsl = gidx0_sb[:, cc0 * 8:cc0 * 8 + ni // 16]
                        else:
                            gb = (co + cc0) * 8 - s0c
                            gsl = gidxR_sb[:, gb:gb + ni // 16]
                        nc.gpsimd.dma_gather(
                            xg[:, cc0:cc0 + n_ch, :],
                            tab_ap,
                            gsl,
                            ni, ni, D, single_packet=False,
                            queue_num=qrr % NQ,
                        )
                        qrr += 1
                        cc0 += n_ch

                # aggregate: identity selection, fp8 DoubleRow over chunk pairs
                pagg = pagg_pool.tile([P, D], F32)
                npair = C // 2
                for k in range(npair):
                    nc.tensor.matmul(
                        pagg[:],
                        lhsT=ident2_sb[:].rearrange("p (two f) -> p two f", two=2),
                        rhs=xg[:, 2 * k:2 * k + 2, :],
                        start=(k == 0),
                        stop=(k == npair - 1 and C % 2 == 0),
                        perf_mode=mybir.MatmulPerfMode.DoubleRow,
                    )
                if C % 2 == 1:
                    nc.tensor.matmul(
                        pagg[:],
                        lhsT=ident2_sb[:, 0:P],
                        rhs=xg[:, C - 1, :],
                        start=(npair == 0),
                        stop=True,
                    )

                # fused PSUM->SBUF copy and dinv[dst] row scale (scalar engine)
                agg_sb = sb_pool.tile([P, D], F32, tag="agg")
                nc.scalar.activation(
                    agg_sb[:], pagg[:], mybir.ActivationFunctionType.Copy,
                    scale=dinv_sb[:, r:r + 1],
                )

                aggT_sb = sb_pool.tile([P, D], BF16, tag="aggT")
                for kb in range(2):
                    pt = pt_pool.tile([P, P], F32)
                    nc.tensor.transpose(
                        pt[:], agg_sb[:, kb * P:(kb + 1) * P], ident_sb[:]
                    )
                    nc.scalar.copy(aggT_sb[:, kb * P:(kb + 1) * P], pt[:])

                pout = pout_pool.tile([P, D], F32)
                nc.tensor.matmul(
                    pout[:], lhsT=aggT_sb[:, 0:P],
                    rhs=wtp0_sb[:], start=True, stop=False,
                )
                nc.tensor.matmul(
                    pout[:], lhsT=aggT_sb[:, P:2 * P],
                    rhs=wtp1_sb[:], start=False, stop=True,
                )

                nc.vector.tensor_tensor(
                    out=out_sb[:, j, :], in0=pout[:], in1=xres_sb[:, j, :],
                    op=mybir.AluOpType.add,
                )
                if j == BAT - 1 or r == SLOTS - 1:
                    r0 = r - j
                    nc.scalar.dma_start(
                        out=out[:, r0 * D:(r + 1) * D].rearrange(
                            "p (b d) -> p b d", b=j + 1
                        ),
                        in_=out_sb[:, 0:j + 1, :],
                    )

    nc.compile()
    return nc


def make_in_maps(inputs):
    """Preprocess + build: returns (nc, in_maps, meta) for run_bass_kernel_spmd."""
    tabs, gidx, xres_sl, dinv_sl, wtp, ident2, ident, meta = _preprocess(**inputs)
    nc = _build(meta)
    bf = mybir.dt.np(BF16)
    in_maps = [
        {
            "xtab0": tabs[0],
            "xtab1": tabs[1],
            "xres": xres_sl[c].reshape(P, SLOTS * D),
            "gidx": gidx[c],
            "dinvv": dinv_sl[c],
            "wtp": wtp.astype(bf),
            "ident2": ident2,
            "ident": ident,
        }
        for c in range(M_CORES)
    ]
    return nc, in_maps, meta


def kernel(node_features, edge_index, W, b, alpha):
    inputs = dict(node_features=node_features, edge_index=edge_index, W=W,
                  b=b, alpha=alpha)
    nc, in_maps, meta = make_in_maps(inputs)
    res = run_bass_kernel_spmd(nc, in_maps, list(range(M_CORES)))
    assign = meta["assign"]
    order = meta["order"]
    outf = np.empty((N_NODES, D), dtype=np.float32)
    for c in range(M_CORES):
        slab = res.results[c]["out"].reshape(P, SLOTS, D)
        for r in range(SLOTS):
            g = int(assign[c, r])
            if g < 0:
                continue
            blk = order[g * P:(g + 1) * P]
            outf[blk] = slab[0:len(blk), r, :]
    return outf


# revision 9
# speedup vs baseline: 1.2004x; 1.2004x over previous
"""GCNConv (PyG-style, alpha-blended residual) on 8 Trainium2 NeuronCores.

Strategy (graph/data parallel, zero collectives):
  out = a*x + (1-a)*(Ahat @ x @ W.T + b)        (aggregate-first form)

v2 redesign (310us -> target ~150us), driven by trace analysis of v1:
  - v1 bottleneck was the dma_gather path: each SWDGE queue serialized
    gen+transfer+sem per ~1280-row call (only ~1 call in flight per queue,
    descriptor in-flight cap ~128/queue at ndesc=rows/16+1), so aggregate
    gather BW averaged 177GB/s vs 347GB/s instantaneous peak. v2 uses
    smaller calls (<=4 chunks = 512 rows, ndesc 33) so 2-3 calls pipeline
    per queue, keeping transfers back-to-back.
  - fp8(e4m3) gather table (dinv[src]-prescaled xs rows, 256B/row): halves
    the dominant DMA stream (53MB -> 29.9MB/core). Numpy-validated final
    rel err ~0.7% vs the 2e-2 gate.
  - Identity selection instead of per-slot DVE S-builds: destination nodes
    are PERMUTED so each group of 128 dsts has near-equal in-degree
    (global (h0,h1) sort) and each source is 2-COLORED into the two
    int16-index table halves so every dst's in-edges split evenly
    (greedy discrepancy minimization). Chunk c of a slot then holds the
    c-th edge of every dst: lane index == dst offset, the selection
    matrix is a constant fp8 identity, holes gather a dedicated zero row
    (9.9% overhead). No S-builds (DVE freed), no dof table, no memsets
    (matmuls only touch gathered chunks).
  - Self-loops are ordinary edges in the gather (src==dst, prescaled
    table makes dinv^2*x automatic): slab2 path of v1 deleted.
  - fp8 DoubleRow PE matmuls aggregate 2 chunks (256-deep contraction)
    per instruction with a stacked-identity lhsT.
  - xres preblended residual in bf16; out fp32; W-chain bf16 as in v1.
Degrees / normalization / coloring / packing are static graph
preprocessing done host-side (pure numpy). NOTE from v1: idx=-1
trailing-trim and bf16 PSUM transposes hang the device - do not
reintroduce.
"""

import numpy as np

import concourse.bacc as bacc
import concourse.bass as bass
import concourse.mybir as mybir
import concourse.tile as tile
from concourse.bass_utils import run_bass_kernel_spmd

N_NODES = 50000
D = 256
M_CORES = 8
P = 128
HALF = 25000
ZROW = HALF  # zero-row index in each half table
NG = (N_NODES + P - 1) // P         # 391 dst groups (permuted)
SLOTS = (NG + M_CORES - 1) // M_CORES  # 49 slots per core
MAX_CALL = 15                       # chunks per call: ndesc 121 <= 128 in-flight cap

F32 = mybir.dt.float32
BF16 = mybir.dt.bfloat16
FP8 = mybir.dt.float8e4
I16 = mybir.dt.int16

NQ = 4                              # SWDGE queues (Q7 core pairs)


def _split_call(c):
    out = []
    while c > 0:
        if c <= MAX_CALL:
            out.append(c)
            break
        take = min(MAX_CALL, (c + 1) // 2)
        out.append(take)
        c -= take
    return out


def _color_sources(srcA, dstA, n):
    """Balanced 2-coloring of sources minimizing per-dst half imbalance."""
    order_by_src = np.argsort(srcA, kind="stable")
    dd = dstA[order_by_src]
    ss = srcA[order_by_src]
    starts = np.searchsorted(ss, np.arange(n + 1))

    rng = np.random.default_rng(0)
    color = np.zeros(n, dtype=np.int8)
    imb = np.zeros(n, dtype=np.int64)  # h0-h1 per dst (assigned so far)
    cnt = [0, 0]
    cap = [HALF, n - HALF]
    proc = rng.permutation(n)
    for s in proc:
        ds = dd[starts[s]:starts[s + 1]]
        sc = imb[ds].sum()
        c = 0 if sc < 0 else 1 if sc > 0 else (0 if cnt[0] <= cnt[1] else 1)
        if cnt[c] >= cap[c]:
            c = 1 - c
        color[s] = c
        cnt[c] += 1
        imb[ds] += 1 if c == 0 else -1
    for _ in range(2):  # refinement
        for s in proc:
            ds = dd[starts[s]:starts[s + 1]]
            sgn = 1 if color[s] == 0 else -1
            if np.sum((imb[ds] - 2 * sgn) ** 2 - imb[ds] ** 2) < 0:
                c = color[s]
                color[s] = 1 - c
                cnt[c] -= 1
                cnt[1 - c] += 1
                imb[ds] -= 2 * sgn
    # restore exact 25000/25000 with least-damaging flips
    over = 0 if cnt[0] > HALF else 1
    sgn = 1 if over == 0 else -1
    while cnt[over] > (HALF if over == 0 else n - HALF):
        cands = np.where(color == over)[0]
        sel = cands if len(cands) <= 4000 else rng.choice(cands, 4000, replace=False)
        best, bestd = None, None
        for s in sel:
            ds = dd[starts[s]:starts[s + 1]]
            dl = np.sum((imb[ds] - 2 * sgn) ** 2 - imb[ds] ** 2)
            if bestd is None or dl < bestd:
                best, bestd = s, dl
        ds = dd[starts[best]:starts[best + 1]]
        color[best] = 1 - over
        cnt[over] -= 1
        cnt[1 - over] += 1
        imb[ds] -= 2 * sgn
    return color


def _preprocess(node_features, edge_index, W, b, alpha):
    x = np.ascontiguousarray(np.asarray(node_features, dtype=np.float32))
    ei = np.asarray(edge_index)
    a = float(np.asarray(alpha).reshape(-1)[0])
    Wf = np.asarray(W, dtype=np.float32)
    bf = np.asarray(b, dtype=np.float32)

    src0 = ei[0].astype(np.int64)
    dst0 = ei[1].astype(np.int64)
    # self-loops as ordinary edges
    srcA = np.concatenate([src0, np.arange(N_NODES)])
    dstA = np.concatenate([dst0, np.arange(N_NODES)])

    deg = np.bincount(dstA, minlength=N_NODES).astype(np.float32)
    dinv = (1.0 / np.sqrt(deg)).astype(np.float32)
    xs = dinv[:, None] * x                      # prescaled gather rows

    # --- source coloring -> table halves -------------------------------
    color = _color_sources(srcA, dstA, N_NODES)
    # table position within half: color-0 nodes by id -> 0..24999, etc.
    pos_in_half = np.zeros(N_NODES, dtype=np.int64)
    for c in (0, 1):
        nodes = np.where(color == c)[0]
        pos_in_half[nodes] = np.arange(len(nodes))
    halfb = color[srcA].astype(np.int64)        # per-edge half
    eidx = pos_in_half[srcA]                    # per-edge idx within half

    fp8 = mybir.dt.np(FP8)
    tabs = []
    for c in (0, 1):
        nodes = np.where(color == c)[0]
        t = np.zeros((HALF + 1, D), dtype=fp8)
        t[:len(nodes)] = xs[nodes].astype(fp8)
        tabs.append(t)                          # row ZROW stays zero

    # --- dst permutation: group by (h0, h1) -----------------------------
    h1 = np.bincount(dstA[halfb == 1], minlength=N_NODES)
    h0 = deg.astype(np.int64) - h1
    order = np.lexsort((h1, h0))                # nodes sorted by (h0, h1)
    C0g = np.zeros(NG, dtype=np.int64)
    C1g = np.zeros(NG, dtype=np.int64)
    for g in range(NG):
        blk = order[g * P:(g + 1) * P]
        C0g[g] = h0[blk].max() if len(blk) else 0
        C1g[g] = h1[blk].max() if len(blk) else 0

    # deal groups 8 per slot row, packing equal (C0,C1) classes together
    gorder = np.argsort(-(C0g * 100 + C1g), kind="stable")
    assign = np.full((M_CORES, SLOTS), -1, dtype=np.int64)
    C0r = np.zeros(SLOTS, dtype=np.int64)
    C1r = np.zeros(SLOTS, dtype=np.int64)
    for r in range(SLOTS):
        blk = gorder[r * M_CORES:(r + 1) * M_CORES]
        for c, g in enumerate(blk):
            assign[c, r] = g
        C0r[r] = C0g[blk].max()
        C1r[r] = C1g[blk].max()
    Cr = C0r + C1r
    cofs = np.concatenate([[0], np.cumsum(Cr)[:-1]])
    TOT = int(Cr.sum())

    # node -> (group, lane); group -> (core, slot)
    grp_of = np.zeros(N_NODES, dtype=np.int64)
    lane_of = np.zeros(N_NODES, dtype=np.int64)
    grp_of[order] = np.arange(N_NODES) // P
    lane_of[order] = np.arange(N_NODES) % P
    core_of_g = np.zeros(NG, dtype=np.int64)
    slot_of_g = np.zeros(NG, dtype=np.int64)
    for c in range(M_CORES):
        for r in range(SLOTS):
            g = assign[c, r]
            if g >= 0:
                core_of_g[g] = c
                slot_of_g[g] = r

    # --- per-edge placement: k-th half-h edge of dst -> chunk k ---------
    # rank edges within (dst, half)
    ekey = dstA * 2 + halfb
    eorder = np.argsort(ekey, kind="stable")
    ks = ekey[eorder]
    kstarts = np.concatenate([[0], np.cumsum(np.bincount(ks, minlength=2 * N_NODES))[:-1]])
    rank = np.arange(len(ks)) - kstarts[ks]     # k within (dst, half)

    ed = dstA[eorder]
    eh = halfb[eorder]
    eix = eidx[eorder]
    g_e = grp_of[ed]
    cr_e = core_of_g[g_e]
    sl_e = slot_of_g[g_e]
    chunk_e = cofs[sl_e] + np.where(eh == 1, C0r[sl_e], 0) + rank
    gpos = chunk_e * P + lane_of[ed]

    idx_arr = np.full((M_CORES, TOT * P), ZROW, dtype=np.int16)  # holes
    idx_arr[cr_e, gpos] = eix.astype(np.int16)

    gidx = [
        np.tile(idx_arr[c].reshape(TOT * 8, 16).T, (8, 1)) for c in range(M_CORES)
    ]

    # --- per-(core,slot) dinv + preblended residual ---------------------
    bf16 = mybir.dt.np(BF16)
    xres_sl = []
    dinv_sl = []
    for c in range(M_CORES):
        slab = np.zeros((P, SLOTS, D), dtype=np.float32)
        dslab = np.zeros((P, SLOTS), dtype=np.float32)
        for r in range(SLOTS):
            g = assign[c, r]
            if g < 0:
                continue
            blk = order[g * P:(g + 1) * P]
            n = len(blk)
            slab[:n, r, :] = a * x[blk] + (1.0 - a) * bf[None, :]
            dslab[:n, r] = dinv[blk]
        xres_sl.append(np.ascontiguousarray(slab.astype(bf16)))
        dinv_sl.append(np.ascontiguousarray(dslab))

    wtp = np.ascontiguousarray(((1.0 - a) * Wf.T).astype(np.float32))
    ident2 = np.concatenate([np.eye(P, dtype=fp8)] * 2, axis=1)  # [P, 2P]
    ident = np.eye(P, dtype=np.float32)

    meta = dict(C0r=C0r, C1r=C1r, cofs=cofs, TOT=TOT, assign=assign, order=order)
    return tabs, gidx, xres_sl, dinv_sl, wtp, ident2, ident, meta


def _build(meta):
    C0r, C1r, cofs, TOT = meta["C0r"], meta["C1r"], meta["cofs"], meta["TOT"]
    nc = bacc.Bacc("TRN2", debug=False, num_swdge_queues=NQ, use_seq_codegen=True)

    xtab0 = nc.dram_tensor("xtab0", [HALF + 1, D], FP8, kind="ExternalInput")
    xtab1 = nc.dram_tensor("xtab1", [HALF + 1, D], FP8, kind="ExternalInput")
    xres = nc.dram_tensor("xres", [P, SLOTS * D], BF16, kind="ExternalInput")
    gidx = nc.dram_tensor("gidx", [P, TOT * 8], I16, kind="ExternalInput")
    dinvv = nc.dram_tensor("dinvv", [P, SLOTS], F32, kind="ExternalInput")
    wtp = nc.dram_tensor("wtp", [2 * P, D], BF16, kind="ExternalInput")
    ident2 = nc.dram_tensor("ident2", [P, 2 * P], FP8, kind="ExternalInput")
    ident = nc.dram_tensor("ident", [P, P], F32, kind="ExternalInput")
    out = nc.dram_tensor("out", [P, SLOTS * D], F32, kind="ExternalOutput")
    BAT = 7                             # slots per xres/out DMA batch

    with tile.TileContext(nc) as tc:
        with (
            tc.tile_pool(name="const", bufs=1) as cpool,
            tc.tile_pool(name="xg", bufs=10) as xg_pool,
            tc.tile_pool(name="sb", bufs=3) as sb_pool,
            tc.tile_pool(name="io", bufs=3) as io_pool,
            tc.tile_pool(name="pagg", bufs=3, space="PSUM") as pagg_pool,
            tc.tile_pool(name="pt", bufs=2, space="PSUM") as pt_pool,
            tc.tile_pool(name="pout", bufs=2, space="PSUM") as pout_pool,
        ):
            # slot-0 index columns first: tiny DMA so gathers start early
            s0c = int(cofs[1]) * 8
            gidx0_sb = cpool.tile([P, s0c], I16)
            gidxR_sb = cpool.tile([P, TOT * 8 - s0c], I16)
            ident2_sb = cpool.tile([P, 2 * P], FP8)
            ident_sb = cpool.tile([P, P], F32)
            wtp0_sb = cpool.tile([P, D], BF16)
            wtp1_sb = cpool.tile([P, D], BF16)
            dinv_sb = cpool.tile([P, SLOTS], F32)
            nc.sync.dma_start(out=gidx0_sb[:], in_=gidx[:, 0:s0c])
            nc.sync.dma_start(out=ident2_sb[:], in_=ident2[:])
            nc.sync.dma_start(out=ident_sb[:], in_=ident[:])
            nc.sync.dma_start(out=wtp0_sb[:], in_=wtp[0:P, :])
            nc.sync.dma_start(out=wtp1_sb[:], in_=wtp[P:2 * P, :])
            nc.sync.dma_start(out=dinv_sb[:], in_=dinvv[:])
            nc.sync.dma_start(out=gidxR_sb[:], in_=gidx[:, s0c:TOT * 8])

            CMAX = int((C0r + C1r).max())

            qrr = 0  # round-robin SWDGE queue over Q7 core pairs
            xres_sb = None
            out_sb = None
            for r in range(SLOTS):
                C0, C1 = int(C0r[r]), int(C1r[r])
                C = C0 + C1
                co = int(cofs[r])
                j = r % BAT
                if j == 0:
                    nb = min(BAT, SLOTS - r)
                    xres_sb = io_pool.tile([P, BAT, D], BF16, tag="xres")
                    nc.scalar.dma_start(
                        out=xres_sb[:, 0:nb, :],
                        in_=xres[:, r * D:(r + nb) * D].rearrange(
                            "p (b d) -> p b d", b=nb
                        ),
                    )
                    out_sb = io_pool.tile([P, BAT, D], F32, tag="out")

                xg = xg_pool.tile([P, CMAX, D], FP8, tag="xg")
                cc0 = 0
                for tab_ap, n_chunks in ((xtab0[:, :], C0), (xtab1[:, :], C1)):
                    for n_ch in _split_call(n_chunks):
                        ni = n_ch * P
                        if r == 0:
                            gsl = gidx0_sb[:, cc0 * 8:cc0 * 8 + ni // 16]
                        else:
                            gb = (co + cc0) * 8 - s0c
                            gsl = gidxR_sb[:, gb:gb + ni // 16]
                        q = qrr % NQ
                        nc.gpsimd.dma_gather(
                            xg[:, cc0:cc0 + n_ch, :],
                            tab_ap,
                            gsl,
                            ni, ni, D, single_packet=False,
                            queue_num=q,
                        )
                        qrr += 1
                        cc0 += n_ch

                # aggregate: identity selection, fp8 DoubleRow over chunk pairs
                pagg = pagg_pool.tile([P, D], F32)
                npair = C // 2
                for k in range(npair):
                    nc.tensor.matmul(
                        pagg[:],
                        lhsT=ident2_sb[:].rearrange("p (two f) -> p two f", two=2),
                        rhs=xg[:, 2 * k:2 * k + 2, :],
                        start=(k == 0),
                        stop=(k == npair - 1 and C % 2 == 0),
                        perf_mode=mybir.MatmulPerfMode.DoubleRow,
                    )
                if C % 2 == 1:
                    nc.tensor.matmul(
                        pagg[:],
                        lhsT=ident2_sb[:, 0:P],
                        rhs=xg[:, C - 1, :],
                        start=(npair == 0),
                        stop=True,
                    )

                # fused PSUM->SBUF copy and dinv[dst] row scale (scalar engine)
                agg_sb = sb_pool.tile([P, D], F32, tag="agg")
                nc.scalar.activation(
                    agg_sb[:], pagg[:], mybir.ActivationFunctionType.Copy,
                    scale=dinv_sb[:, r:r + 1],
                )

                aggT_sb = sb_pool.tile([P, D], BF16, tag="aggT")
                for kb in range(2):
                    pt = pt_pool.tile([P, P], F32)
                    nc.tensor.transpose(
                        pt[:], agg_sb[:, kb * P:(kb + 1) * P], ident_sb[:]
                    )
                    nc.scalar.copy(aggT_sb[:, kb * P:(kb + 1) * P], pt[:])

                pout = pout_pool.tile([P, D], F32)
                nc.tensor.matmul(
                    pout[:], lhsT=aggT_sb[:, 0:P],
                    rhs=wtp0_sb[:], start=True, stop=False,
                )
                nc.tensor.matmul(
                    pout[:], lhsT=aggT_sb[:, P:2 * P],
                    rhs=wtp1_sb[:], start=False, stop=True,
                )

                nc.vector.tensor_tensor(
                    out=out_sb[:, j, :], in0=pout[:], in1=xres_sb[:, j, :],
                    op=mybir.AluOpType.add,
                )
                if j == BAT - 1 or r == SLOTS - 1:
                    r0 = r - j
                    nc.scalar.dma_start(
                        out=out[:, r0 * D:(r + 1) * D].rearrange(
                            "p (b d) -> p b d", b=j + 1
                        ),
                        in_=out_sb[:, 0:j + 1, :],
                    )

    nc.compile()
    return nc


def make_in_maps(inputs):
    """Preprocess + build: returns (nc, in_maps, meta) for run_bass_kernel_spmd."""
    tabs, gidx, xres_sl, dinv_sl, wtp, ident2, ident, meta = _preprocess(**inputs)
    nc = _build(meta)
    bf = mybir.dt.np(BF16)
    in_maps = [
        {
            "xtab0": tabs[0],
            "xtab1": tabs[1],
            "xres": xres_sl[c].reshape(P, SLOTS * D),
            "gidx": gidx[c],
            "dinvv": dinv_sl[c],
            "wtp": wtp.astype(bf),
            "ident2": ident2,
            "ident": ident,
        }
        for c in range(M_CORES)
    ]
    return nc, in_maps, meta


def kernel(node_features, edge_index, W, b, alpha):
    inputs = dict(node_features=node_features, edge_index=edge_index, W=W,
                  b=b, alpha=alpha)
    nc, in_maps, meta = make_in_maps(inputs)
    res = run_bass_kernel_spmd(nc, in_maps, list(range(M_CORES)))
    assign = meta["assign"]
    order = meta["order"]
    outf = np.empty((N_NODES, D), dtype=np.float32)
    for c in range(M_CORES):
        slab = res.results[c]["out"].reshape(P, SLOTS, D)
        for r in range(SLOTS):
            g = int(assign[c, r])
            if g < 0:
                continue
            blk = order[g * P:(g + 1) * P]
            outf[blk] = slab[0:len(blk), r, :]
    return outf
